# revision 1
# baseline (speedup 1.0000x reference)
"""Trainium2 Bass kernel for nn_Loss_343597383760.

Loss:
    scores = predicted_values[rel_idx, e1_idx, e2_idx]        # [N] gather
    sig    = sigmoid(scores)
    total  = sum(lab*sig + (1-lab)*(1-sig)) = neg + sum(w*sig),  w = 2*lab-1
    loss   = -total / ((1+neg)*N)

Sharding (expert-style, per relation): core c owns relations {2c, 2c+1} of
predicted_values ([2,4096,4096] f32 = 128 MiB per core). Host buckets the
262144 triplets by owning core, converts each to a flat element index into
the local shard, pads each bucket to a fixed capacity CAP, and ships one
[128, 2*COLS] int32 "meta" plane per core: idx columns then f32-bitcast
weight columns (w in {+1,-1}, 0 for pads).

Device per core, chunked pipeline:
    indirect DMA gather (4B/elem, SWDGE)      g   = pv[idx]
    DVE multiply                              t   = w * g
    ACT sigmoid with accum_out                out[:,k] = sum_row sigmoid(t)
Host uses sum sig(w*s) = sum w*sig(s) + neg_c + 0.5*pad_c (per core) to
recover the weighted sum exactly, then forms the scalar loss.
"""

import numpy as np

import concourse.bass as bass
import concourse.bacc as bacc
import concourse.tile as tile
from concourse import mybir
from concourse.bass_utils import run_bass_kernel_spmd

R, E, N = 16, 4096, 262144
NCORES = 8
RPC = R // NCORES            # relations per core
TOTAL = RPC * E * E          # elements in one core's shard
P = 128                      # SBUF partitions
COLS = 264                   # capacity per core = 128*264 = 33792 (max bucket 33040)
CAP = P * COLS
NCHUNK = 4
CCH = COLS // NCHUNK
NQ = 2                       # SWDGE queues; gathers alternate between them

# Set by test harness to capture a neuron-profile trace.
TRACE = False
LAST_RESULTS = None

_NC = None


def _indirect_gather_q(nc, out, in_, in_offset, queue_name):
    """indirect_dma_start with an explicit SWDGE queue (the stock API pins
    qPoolDynamic; alternating queues lets ring drains overlap desc-gen)."""
    orig = mybir.InstDMACopy

    def patched(**kw):
        kw["queue"] = queue_name
        return orig(**kw)

    mybir.InstDMACopy = patched
    try:
        return nc.gpsimd.indirect_dma_start(
            out=out, out_offset=None, in_=in_, in_offset=in_offset
        )
    finally:
        mybir.InstDMACopy = orig


def _build_nc():
    f32 = mybir.dt.float32
    i32 = mybir.dt.int32
    nc = bacc.Bacc(num_swdge_queues=NQ)
    pv = nc.declare_dram_parameter("pv", [TOTAL, 1], f32, isOutput=False)
    idxs = nc.declare_dram_parameter("idx", [P, COLS], i32, isOutput=False)
    wts = nc.declare_dram_parameter("wts", [P, COLS], f32, isOutput=False)
    out = nc.declare_dram_parameter("out", [P, NCHUNK], f32, isOutput=True)

    with (
        tile.TileContext(nc) as tc,
        tc.tile_pool(name="io", bufs=1) as io_pool,
        tc.tile_pool(name="work", bufs=4) as work_pool,
        tc.tile_pool(name="res", bufs=1) as res_pool,
    ):
        it = io_pool.tile([P, COLS], i32, tag="idx")
        nc.sync.dma_start(out=it[:], in_=idxs[:])
        wt = io_pool.tile([P, COLS], f32, tag="wts")
        nc.scalar.dma_start(out=wt[:], in_=wts[:])
        outbuf = res_pool.tile([P, NCHUNK], f32)
        for k in range(NCHUNK):
            sl = slice(k * CCH, (k + 1) * CCH)
            g = work_pool.tile([P, CCH], f32, tag="gath")
            _indirect_gather_q(
                nc,
                out=g[:],
                in_=pv[:],
                in_offset=bass.IndirectOffsetOnAxis(ap=it[:, sl], axis=0),
                queue_name=f"qPoolDynamic{k % NQ or ''}",
            )
            t = work_pool.tile([P, CCH], f32, tag="wprod")
            nc.vector.tensor_tensor(
                out=t[:],
                in0=g[:],
                in1=wt[:, sl],
                op=mybir.AluOpType.mult,
            )
            sg = work_pool.tile([P, CCH], f32, tag="sig")
            nc.scalar.activation(
                out=sg[:],
                in_=t[:],
                func=mybir.ActivationFunctionType.Sigmoid,
                accum_out=outbuf[:, k : k + 1],
            )
        nc.sync.dma_start(out=out[:], in_=outbuf[:])
    nc.finalize()
    return nc


def kernel(predicted_values, rel_idx, e1_idx, e2_idx, labels):
    global _NC, LAST_RESULTS
    pv = np.ascontiguousarray(np.asarray(predicted_values, dtype=np.float32))
    rel = np.asarray(rel_idx, dtype=np.int64)
    e1 = np.asarray(e1_idx, dtype=np.int64)
    e2 = np.asarray(e2_idx, dtype=np.int64)
    lab = np.asarray(labels, dtype=np.int64)

    owner = rel // RPC
    local_flat = (rel % RPC) * (E * E) + e1 * E + e2  # < TOTAL, fits int32
    w = (2 * lab - 1).astype(np.float32)

    pv_flat = pv.reshape(R * E * E)
    host_extra = 0.0   # sum of w*sig for overflow triplets (host-computed)
    correction = 0.0   # sum over cores of (neg_c + 0.5*pad_c)
    in_maps = []
    for c in range(NCORES):
        m = owner == c
        fi = local_flat[m]
        wi = w[m]
        if fi.size > CAP:
            of = fi[CAP:] + c * TOTAL
            ow = wi[CAP:].astype(np.float64)
            s = pv_flat[of].astype(np.float64)
            host_extra += float(np.sum(ow / (1.0 + np.exp(-s))))
            fi = fi[:CAP]
            wi = wi[:CAP]
        neg_c = float(np.sum(wi < 0.0))
        pad_c = float(CAP - fi.size)
        correction += neg_c + 0.5 * pad_c
        idx_arr = np.zeros(CAP, np.int32)
        idx_arr[: fi.size] = fi.astype(np.int32)
        w_arr = np.zeros(CAP, np.float32)
        w_arr[: wi.size] = wi
        in_maps.append(
            {
                "pv": pv[c * RPC : (c + 1) * RPC].reshape(TOTAL, 1),
                "idx": idx_arr.reshape(P, COLS),
                "wts": w_arr.reshape(P, COLS),
            }
        )

    if _NC is None:
        _NC = _build_nc()

    res = run_bass_kernel_spmd(
        _NC, in_maps, core_ids=list(range(NCORES)), trace=TRACE
    )
    LAST_RESULTS = res

    # device sums sig(w*s) per slot; sum w*sig(s) = dev_sum - neg_c - 0.5*pad_c
    asig = host_extra - correction
    for c in range(NCORES):
        asig += float(np.asarray(res.results[c]["out"], dtype=np.float64).sum())

    neg = float(np.sum(lab == 0))
    loss = -(neg + asig) / ((1.0 + neg) * float(N))
    return np.array([loss], dtype=np.float32)



# revision 2
# speedup vs baseline: 1.0129x; 1.0129x over previous
"""Trainium2 Bass kernel for nn_Loss_343597383760.

Loss:
    scores = predicted_values[rel_idx, e1_idx, e2_idx]        # [N] gather
    sig    = sigmoid(scores)
    total  = sum(lab*sig + (1-lab)*(1-sig)) = neg + sum(w*sig),  w = 2*lab-1
    loss   = -total / ((1+neg)*N)

Sharding (expert-style, per relation): core c owns relations {2c, 2c+1} of
predicted_values ([2,4096,4096] f32 = 128 MiB per core). Host buckets the
262144 triplets by owning core, converts each to a flat element index into
the local shard, pads each bucket to a fixed capacity CAP, and ships one
[128, COLS] int32 idx plane and one [128, COLS] f32 weight plane per core
(w in {+1,-1}, 0 for pads).

Device per core, chunked pipeline with per-chunk idx loads so the first
gather starts as soon as its own slice of indices lands in SBUF:
    per chunk k:  idx load (HWDGE)  ->  indirect SWDGE gather g = pv[idx]
                  DVE multiply t = w*g  ->  ACT sigmoid accum_out
Chunk sizes are uneven: small first chunk (earliest possible gather
dispatch), large middle chunks (amortize the per-instruction descriptor-
generation overhead on the Pool sequencer), tiny last chunk (minimize the
exposed tail drain + epilogue).
Host uses sum sig(w*s) = sum w*sig(s) + neg_c + 0.5*pad_c (per core) to
recover the weighted sum exactly, then forms the scalar loss.
"""

import numpy as np

import concourse.bass as bass
import concourse.bacc as bacc
import concourse.tile as tile
from concourse import mybir
from concourse.bass_utils import run_bass_kernel_spmd

R, E, N = 16, 4096, 262144
NCORES = 8
RPC = R // NCORES            # relations per core
TOTAL = RPC * E * E          # elements in one core's shard
P = 128                      # SBUF partitions

# Chunk schedule, in columns of the [128, COLS] index/weight planes.
CHUNKS = [40, 112, 96, 16]
COLS = sum(CHUNKS)           # capacity per core = 128*264 = 33792 (max bucket 33040)
CAP = P * COLS
NQ = 2                       # SWDGE queues; gathers alternate between them

# Set by test harness to capture a neuron-profile trace.
TRACE = False
LAST_RESULTS = None

_NC = None


def _indirect_gather_q(nc, out, in_, in_offset, queue_name):
    """indirect_dma_start with an explicit SWDGE queue (the stock API pins
    qPoolDynamic; alternating queues lets ring drains overlap desc-gen)."""
    orig = mybir.InstDMACopy

    def patched(**kw):
        kw["queue"] = queue_name
        return orig(**kw)

    mybir.InstDMACopy = patched
    try:
        return nc.gpsimd.indirect_dma_start(
            out=out, out_offset=None, in_=in_, in_offset=in_offset
        )
    finally:
        mybir.InstDMACopy = orig


def _build_nc():
    f32 = mybir.dt.float32
    i32 = mybir.dt.int32
    nchunk = len(CHUNKS)
    nc = bacc.Bacc(num_swdge_queues=NQ)
    pv = nc.declare_dram_parameter("pv", [TOTAL, 1], f32, isOutput=False)
    idxs = nc.declare_dram_parameter("idx", [P, COLS], i32, isOutput=False)
    wts = nc.declare_dram_parameter("wts", [P, COLS], f32, isOutput=False)
    out = nc.declare_dram_parameter("out", [P, nchunk], f32, isOutput=True)

    with (
        tile.TileContext(nc) as tc,
        tc.tile_pool(name="io", bufs=1) as io_pool,
        tc.tile_pool(name="work", bufs=4) as work_pool,
        tc.tile_pool(name="res", bufs=1) as res_pool,
    ):
        outbuf = res_pool.tile([P, nchunk], f32)
        its = []
        off = 0
        # issue all per-chunk idx loads up front (sync HWDGE queue drains
        # them in order; chunk 0 is small so its completion sem fires early)
        for k, c in enumerate(CHUNKS):
            it = io_pool.tile([P, c], i32, tag=f"idx{k}")
            nc.sync.dma_start(out=it[:], in_=idxs[:, off : off + c])
            its.append(it)
            off += c
        wt = io_pool.tile([P, COLS], f32, tag="wts")
        nc.scalar.dma_start(out=wt[:], in_=wts[:])
        off = 0
        for k, c in enumerate(CHUNKS):
            g = work_pool.tile([P, c], f32, tag=f"gath{k}")
            _indirect_gather_q(
                nc,
                out=g[:],
                in_=pv[:],
                in_offset=bass.IndirectOffsetOnAxis(ap=its[k][:], axis=0),
                queue_name=f"qPoolDynamic{k % NQ or ''}",
            )
            t = work_pool.tile([P, c], f32, tag=f"wprod{k}")
            nc.vector.tensor_tensor(
                out=t[:],
                in0=g[:],
                in1=wt[:, off : off + c],
                op=mybir.AluOpType.mult,
            )
            sg = work_pool.tile([P, c], f32, tag=f"sig{k}")
            nc.scalar.activation(
                out=sg[:],
                in_=t[:],
                func=mybir.ActivationFunctionType.Sigmoid,
                accum_out=outbuf[:, k : k + 1],
            )
            off += c
        nc.sync.dma_start(out=out[:], in_=outbuf[:])
    nc.finalize()
    return nc


def kernel(predicted_values, rel_idx, e1_idx, e2_idx, labels):
    global _NC, LAST_RESULTS
    pv = np.ascontiguousarray(np.asarray(predicted_values, dtype=np.float32))
    rel = np.asarray(rel_idx, dtype=np.int64)
    e1 = np.asarray(e1_idx, dtype=np.int64)
    e2 = np.asarray(e2_idx, dtype=np.int64)
    lab = np.asarray(labels, dtype=np.int64)

    owner = rel // RPC
    local_flat = (rel % RPC) * (E * E) + e1 * E + e2  # < TOTAL, fits int32
    w = (2 * lab - 1).astype(np.float32)

    pv_flat = pv.reshape(R * E * E)
    host_extra = 0.0   # sum of w*sig for overflow triplets (host-computed)
    correction = 0.0   # sum over cores of (neg_c + 0.5*pad_c)
    in_maps = []
    for c in range(NCORES):
        m = owner == c
        fi = local_flat[m]
        wi = w[m]
        if fi.size > CAP:
            of = fi[CAP:] + c * TOTAL
            ow = wi[CAP:].astype(np.float64)
            s = pv_flat[of].astype(np.float64)
            host_extra += float(np.sum(ow / (1.0 + np.exp(-s))))
            fi = fi[:CAP]
            wi = wi[:CAP]
        neg_c = float(np.sum(wi < 0.0))
        pad_c = float(CAP - fi.size)
        correction += neg_c + 0.5 * pad_c
        idx_arr = np.zeros(CAP, np.int32)
        idx_arr[: fi.size] = fi.astype(np.int32)
        w_arr = np.zeros(CAP, np.float32)
        w_arr[: wi.size] = wi
        in_maps.append(
            {
                "pv": pv[c * RPC : (c + 1) * RPC].reshape(TOTAL, 1),
                "idx": idx_arr.reshape(P, COLS),
                "wts": w_arr.reshape(P, COLS),
            }
        )

    if _NC is None:
        _NC = _build_nc()

    res = run_bass_kernel_spmd(
        _NC, in_maps, core_ids=list(range(NCORES)), trace=TRACE
    )
    LAST_RESULTS = res

    # device sums sig(w*s) per slot; sum w*sig(s) = dev_sum - neg_c - 0.5*pad_c
    asig = host_extra - correction
    for c in range(NCORES):
        asig += float(np.asarray(res.results[c]["out"], dtype=np.float64).sum())

    neg = float(np.sum(lab == 0))
    loss = -(neg + asig) / ((1.0 + neg) * float(N))
    return np.array([loss], dtype=np.float32)


# revision 5
# speedup vs baseline: 1.1145x; 1.1002x over previous
"""Trainium2 Bass kernel for nn_Loss_343597383760.

Loss:
    scores = predicted_values[rel_idx, e1_idx, e2_idx]        # [N] gather
    sig    = sigmoid(scores)
    total  = sum(lab*sig + (1-lab)*(1-sig)) = neg + sum(w*sig),  w = 2*lab-1
    loss   = -total / ((1+neg)*N)

Sharding (expert-style, per relation): core c owns relations {2c, 2c+1} of
predicted_values ([2,4096,4096] f32 = 128 MiB per core). Host buckets the
262144 triplets by owning core, converts each to a flat element index into
the local shard, pads each bucket to a fixed capacity CAP, and ships one
[128, COLS] int32 idx plane and one [128, COLS] f32 weight plane per core
(w in {+1,-1}, 0 for pads).

Device per core, chunked pipeline with per-chunk idx loads so the first
gather starts as soon as its own slice of indices lands in SBUF:
    per chunk k:  idx load (HWDGE)  ->  indirect SWDGE gather g = pv[idx]
                  DVE multiply t = w*g  ->  ACT sigmoid accum_out
Chunk sizes are uneven: small first chunk (earliest possible gather
dispatch), large middle chunks (amortize the per-instruction descriptor-
generation overhead on the Pool sequencer), tiny last chunk (minimize the
exposed tail drain + epilogue).
Host uses sum sig(w*s) = sum w*sig(s) + neg_c + 0.5*pad_c (per core) to
recover the weighted sum exactly, then forms the scalar loss.
"""

import numpy as np

import concourse.bass as bass
import concourse.bacc as bacc
import concourse.tile as tile
from concourse import mybir
from concourse.bass_utils import run_bass_kernel_spmd

R, E, N = 16, 4096, 262144
NCORES = 8
RPC = R // NCORES            # relations per core
TOTAL = RPC * E * E          # elements in one core's shard
P = 128                      # SBUF partitions

# Chunk schedule, in columns of the [128, COLS] index/weight planes.
# First chunk small (its idx plane ships via a dedicated early DMA so the
# first gather dispatches ASAP); last chunk small (short exposed tail).
CHUNKS = [16, 232, 16]
COLS = sum(CHUNKS)           # capacity per core = 128*264 = 33792 (max bucket 33040)
CAP = P * COLS
NQ = 2                       # SWDGE queues; gathers alternate between them

# Set by test harness to capture a neuron-profile trace.
TRACE = False
LAST_RESULTS = None

_NC = None


def _indirect_gather_q(nc, out, in_, in_offset, queue_name):
    """indirect_dma_start with an explicit SWDGE queue (the stock API pins
    qPoolDynamic; alternating queues lets ring drains overlap desc-gen)."""
    orig = mybir.InstDMACopy

    def patched(**kw):
        kw["queue"] = queue_name
        return orig(**kw)

    mybir.InstDMACopy = patched
    try:
        return nc.gpsimd.indirect_dma_start(
            out=out, out_offset=None, in_=in_, in_offset=in_offset
        )
    finally:
        mybir.InstDMACopy = orig


def _build_nc():
    f32 = mybir.dt.float32
    i32 = mybir.dt.int32
    nchunk = len(CHUNKS)
    nc = bacc.Bacc(num_swdge_queues=NQ)
    pv = nc.declare_dram_parameter("pv", [TOTAL, 1], f32, isOutput=False)
    idxs = nc.declare_dram_parameter("idx", [P, COLS], i32, isOutput=False)
    wts = nc.declare_dram_parameter("wts", [P, COLS], f32, isOutput=False)
    out = nc.declare_dram_parameter("out", [P, nchunk], f32, isOutput=True)

    with (
        tile.TileContext(nc) as tc,
        tc.tile_pool(name="io", bufs=1) as io_pool,
        tc.tile_pool(name="work", bufs=4) as work_pool,
        tc.tile_pool(name="res", bufs=1) as res_pool,
    ):
        outbuf = res_pool.tile([P, nchunk], f32)
        c0 = CHUNKS[0]
        # chunk-0 idx plane on the SP HWDGE queue (small -> earliest
        # completion sem -> earliest first gather); the rest of the idx
        # plane on the ACT HWDGE queue so the completion receipts don't
        # serialize behind one ring; weights queued behind chunk-0 idx.
        it0 = io_pool.tile([P, c0], i32, tag="idx0")
        nc.sync.dma_start(out=it0[:], in_=idxs[:, :c0])
        itr = io_pool.tile([P, COLS - c0], i32, tag="idxr")
        nc.scalar.dma_start(out=itr[:], in_=idxs[:, c0:])
        wt = io_pool.tile([P, COLS], f32, tag="wts")
        nc.sync.dma_start(out=wt[:], in_=wts[:])
        its = [it0[:]]
        off = 0
        for c in CHUNKS[1:]:
            its.append(itr[:, off : off + c])
            off += c
        off = 0
        for k, c in enumerate(CHUNKS):
            g = work_pool.tile([P, c], f32, tag=f"gath{k}")
            _indirect_gather_q(
                nc,
                out=g[:],
                in_=pv[:],
                in_offset=bass.IndirectOffsetOnAxis(ap=its[k], axis=0),
                queue_name=f"qPoolDynamic{k % NQ or ''}",
            )
            t = work_pool.tile([P, c], f32, tag=f"wprod{k}")
            nc.vector.tensor_tensor(
                out=t[:],
                in0=g[:],
                in1=wt[:, off : off + c],
                op=mybir.AluOpType.mult,
            )
            sg = work_pool.tile([P, c], f32, tag=f"sig{k}")
            nc.scalar.activation(
                out=sg[:],
                in_=t[:],
                func=mybir.ActivationFunctionType.Sigmoid,
                accum_out=outbuf[:, k : k + 1],
            )
            off += c
        nc.sync.dma_start(out=out[:], in_=outbuf[:])
    nc.finalize()
    return nc


def kernel(predicted_values, rel_idx, e1_idx, e2_idx, labels):
    global _NC, LAST_RESULTS
    pv = np.ascontiguousarray(np.asarray(predicted_values, dtype=np.float32))
    rel = np.asarray(rel_idx, dtype=np.int64)
    e1 = np.asarray(e1_idx, dtype=np.int64)
    e2 = np.asarray(e2_idx, dtype=np.int64)
    lab = np.asarray(labels, dtype=np.int64)

    owner = rel // RPC
    local_flat = (rel % RPC) * (E * E) + e1 * E + e2  # < TOTAL, fits int32
    w = (2 * lab - 1).astype(np.float32)

    pv_flat = pv.reshape(R * E * E)
    host_extra = 0.0   # sum of w*sig for overflow triplets (host-computed)
    correction = 0.0   # sum over cores of (neg_c + 0.5*pad_c)
    in_maps = []
    for c in range(NCORES):
        m = owner == c
        fi = local_flat[m]
        wi = w[m]
        if fi.size > CAP:
            of = fi[CAP:] + c * TOTAL
            ow = wi[CAP:].astype(np.float64)
            s = pv_flat[of].astype(np.float64)
            host_extra += float(np.sum(ow / (1.0 + np.exp(-s))))
            fi = fi[:CAP]
            wi = wi[:CAP]
        neg_c = float(np.sum(wi < 0.0))
        pad_c = float(CAP - fi.size)
        correction += neg_c + 0.5 * pad_c
        idx_arr = np.zeros(CAP, np.int32)
        idx_arr[: fi.size] = fi.astype(np.int32)
        w_arr = np.zeros(CAP, np.float32)
        w_arr[: wi.size] = wi
        in_maps.append(
            {
                "pv": pv[c * RPC : (c + 1) * RPC].reshape(TOTAL, 1),
                "idx": idx_arr.reshape(P, COLS),
                "wts": w_arr.reshape(P, COLS),
            }
        )

    if _NC is None:
        _NC = _build_nc()

    res = run_bass_kernel_spmd(
        _NC, in_maps, core_ids=list(range(NCORES)), trace=TRACE
    )
    LAST_RESULTS = res

    # device sums sig(w*s) per slot; sum w*sig(s) = dev_sum - neg_c - 0.5*pad_c
    asig = host_extra - correction
    for c in range(NCORES):
        asig += float(np.asarray(res.results[c]["out"], dtype=np.float64).sum())

    neg = float(np.sum(lab == 0))
    loss = -(neg + asig) / ((1.0 + neg) * float(N))
    return np.array([loss], dtype=np.float32)
